# revision 31
# baseline (speedup 1.0000x reference)
"""GQA causal attention (S=2048, H=32, KVH=8, D=128) on 8 TRN2 NeuronCores.

Sharding: tensor-parallel over heads. Core i computes query heads
[4i, 4i+4) against KV head i (GQA group size 32/8 = 4). No collectives:
the host slices the inputs per core and concatenates the outputs.

Per-core algorithm (seq=2048, d=128, 4 q-heads, 1 kv-head, causal):
  - Q is loaded whole (all 4 heads) in contiguous 2KB-row DMA chunks --
    4x fewer/larger descriptors than per-head strided loads. K/V load in
    512B-row chunks on the other HWDGE ring. Heads are cast to bf16 on
    the DVE and transposed to [d=128, seq] by PE identity matmuls,
    spread through the previous head's compute.
  - Per head, exact-causal score tiles S^T[kt] = K_tile^T @ Q^T (only
    q >= kt*128) are written PACKED into PSUM buffers B[128,1024] /
    A[128,2048] (B first, so each head's first ACTIVATE has a short
    dependency); ONE wide ACTIVATE(Exp, scale) per buffer writes the
    packed P^T row [128, 17408] bf16 (scores are O(1), so no max
    subtraction). 44 activations instead of 96 -- the scalar engine is
    the steady-state bottleneck at (cols + 352)/1.2GHz per activation.
  - The diagonal 128-col block of each key-tile region is masked by a
    0/1 upper-triangular multiply on the (otherwise idle) GpSimd engine.
  - PV: for each query tile qt, acc[qt] = sum_k2 (P^T slice).T @ [V | 1]
    accumulated in PSUM (3 rotating slices so the DVE normalize never
    stalls the chain); column 128 is the softmax denominator. DVE
    reciprocal + tensor_scalar_mul normalizes; one DMA per 256 rows
    stores the result. PV lags the QK/exp pipeline by a few query tiles
    and flows across head boundaries.
  - One dummy 129-col matmul per buffer cycle parks in a spare PSUM slot
    purely to keep the HAM clock-gate from re-throttling the PE to
    1.2 GHz during scalar-bound stretches.
"""

import numpy as np

SEQ = 2048
D = 128
QH = 4  # query heads per core
N_CORES = 8
SCALE = 0.08838834764831845  # 1/sqrt(128)
NT = SEQ // 128  # 16 tiles of 128 along seq

_NC = None

# packed score-column layout (identical per head)
ROFF = [0]
for _kt in range(1, NT + 1):
    ROFF.append(ROFF[-1] + (SEQ - 128 * (_kt - 1)))
PCOLS = ROFF[NT]  # 17408

# psum buffers: B(1024) first so the head's first ACTIVATE has a short
# dependency chain, then alternate with A(2048); the tail is one extra A.
_SIZES = [1024, 2048] * 5 + [2048]  # sums to PCOLS
BUFS = []
_c = 0
for _sz in _SIZES:
    BUFS.append((_c, _sz, 1 if _sz == 1024 else 0))  # (start, size, pool: 0=A,1=B)
    _c += _sz
assert _c == PCOLS


def _emit(ctx, tc, q, k, v, out):
    import concourse.mybir as mybir
    from concourse import masks

    nc = tc.nc
    f32 = mybir.dt.float32
    bf16 = mybir.dt.bfloat16
    Exp = mybir.ActivationFunctionType.Exp

    singles = ctx.enter_context(tc.tile_pool(name="singles", bufs=1))
    ppool = ctx.enter_context(tc.tile_pool(name="ppool", bufs=2))
    opool = ctx.enter_context(tc.tile_pool(name="opool", bufs=3))
    qbfp = ctx.enter_context(tc.tile_pool(name="qbfp", bufs=2))
    # PSUM budget (8 banks = 16KB/partition):
    #   A 2048 f32 = 4 banks, B 1024 f32 = 2 banks,
    #   PV acc [128,3,129] f32 = 1 bank, transpose+warm staging = 1 bank
    psum_a = ctx.enter_context(tc.tile_pool(name="psum_a", bufs=1, space="PSUM"))
    psum_b = ctx.enter_context(tc.tile_pool(name="psum_b", bufs=1, space="PSUM"))
    psum_o = ctx.enter_context(tc.tile_pool(name="psum_o", bufs=1, space="PSUM"))
    psum_t = ctx.enter_context(tc.tile_pool(name="psum_t", bufs=1, space="PSUM"))

    sA = psum_a.tile([128, 2048], f32, tag="A")
    sB = psum_b.tile([128, 1024], f32, tag="B")
    ops_tri = psum_o.tile([128, 3, D + 1], f32, tag="o")
    # two transpose staging slots inside one PSUM bank (slices rotate)
    tps = psum_t.tile([128, 2, 128], bf16, tag="tp")

    # ---- PE warmup: HAM needs ~3.4us of continuous matmul activity to
    # lift the clock gate to 2.4 GHz; identity transposes don't count.
    warm_src = singles.tile([128, 512], bf16, tag="warm_src")
    nc.vector.memset(warm_src[:], 0.0)

    def warm(n):
        # dummies park in PV slot 0; prep warms are ordered against any
        # overlapping PV chains by the tile framework (correctness-safe)
        for _ in range(n):
            nc.tensor.matmul(
                ops_tri[:, 0, :], lhsT=warm_src[:, 0:128],
                rhs=warm_src[:, 0:D + 1], start=True, stop=True,
            )

    warm(28)

    ident = singles.tile([128, 128], bf16)
    masks.make_identity(nc, ident[:])
    keep = singles.tile([128, 128], bf16)
    masks.make_upper_triangular(nc, keep[:], val=1.0, diag=True)

    # ---- loads: whole Q (all heads) in contiguous 1MB chunks on the
    # scalar HWDGE ring; K/V (512B-row) chunks on the sync ring.
    qnat = singles.tile([128, NT, QH * D], f32, tag="qnat")
    qr = q.rearrange("(t p) d -> p t d", p=128)
    qld = [False] * 4

    def need_qld(c):
        if not qld[c]:
            qld[c] = True
            cs = slice(c * 4, (c + 1) * 4)
            nc.scalar.dma_start(out=qnat[:, cs, :], in_=qr[:, cs, :])

    kT = singles.tile([128, SEQ], bf16, tag="kT")
    knat = singles.tile([128, NT, 128], f32, tag="knat")
    knat_bf = singles.tile([128, NT, 128], bf16, tag="knat_bf")
    kr = k.rearrange("(t p) d -> p t d", p=128)
    vp = singles.tile([128, NT, D + 1], bf16)
    vnat = singles.tile([128, NT, 128], f32, tag="vnat")
    vr = v.rearrange("(t p) d -> p t d", p=128)
    nc.vector.memset(vp[:, :, D:D + 1], 1.0)

    qT = [
        singles.tile([128, SEQ], bf16, tag=f"qT{h}", name=f"qT{h}")
        for h in range(QH)
    ]

    def qprep_chunk(h, c):
        """Cast + PE-transpose one 4-tile chunk of head h's Q."""
        need_qld(c)
        cs = slice(c * 4, (c + 1) * 4)
        qbf = qbfp.tile([128, 4, 128], bf16, tag="qbf", name="qbf")
        nc.vector.tensor_copy(qbf[:], qnat[:, cs, h * D:(h + 1) * D])
        for t in range(4):
            pst = tps[:, t % 2, :]
            nc.tensor.transpose(pst, qbf[:, t, :], ident[:])
            nc.vector.tensor_copy(
                qT[h][:, (c * 4 + t) * 128:(c * 4 + t + 1) * 128], pst
            )
            if t % 2 == 1:
                warm(1)

    def kchunk(c):
        cs = slice(c * 4, (c + 1) * 4)
        nc.sync.dma_start(out=knat[:, cs, :], in_=kr[:, cs, :])
        nc.vector.tensor_copy(knat_bf[:, cs, :], knat[:, cs, :])
        for t in range(c * 4, (c + 1) * 4):
            pst = tps[:, t % 2, :]
            nc.tensor.transpose(pst, knat_bf[:, t, :], ident[:])
            nc.vector.tensor_copy(kT[:, t * 128:(t + 1) * 128], pst)
            if t % 2 == 1:
                warm(1)

    def vchunk(c):
        cs = slice(c * 8, (c + 1) * 8)
        nc.sync.dma_start(out=vnat[:, cs, :], in_=vr[:, cs, :])
        nc.vector.tensor_copy(vp[:, cs, 0:D], vnat[:, cs, :])

    # Lazy head-0 prep, emitted just-in-time from inside the buffer walk
    prep_state = {"k": 0, "q0": 0}

    def need_k(kt):
        while prep_state["k"] * 4 <= kt:
            c = prep_state["k"]
            kchunk(c)
            warm(1)
            if c < 2:
                vchunk(c)
            prep_state["k"] += 1

    def need_q0(qhi):
        while prep_state["q0"] * 512 < qhi:
            qprep_chunk(0, prep_state["q0"])
            warm(1)
            prep_state["q0"] += 1

    # spread the next head's Q prep over the current head's buffers
    QPREP_EVENTS = {}
    for _h in range(1, QH):
        for _c in range(4):
            QPREP_EVENTS[(_h - 1, 3 + 2 * _c)] = (_h, _c)

    def emit_pv(h, qt, pT, osb):
        """O[qt] = sum_k2 (P^T slice).T @ [V | 1], then normalize + store."""
        ops = ops_tri[:, qt % 3, :]
        for k2 in range(qt + 1):
            c0 = ROFF[k2] + (qt - k2) * 128
            nc.tensor.matmul(
                ops,
                lhsT=pT[:, c0:c0 + 128],
                rhs=vp[:, k2, :],
                start=(k2 == 0),
                stop=(k2 == qt),
            )
        rec = opool.tile([128, 1], f32, tag="rec")
        nc.vector.reciprocal(rec[:], ops[:, D:D + 1])
        nc.vector.tensor_scalar_mul(osb[:, qt % 2, :], ops[:, 0:D], rec[:])
        if qt % 2 == 1:
            qb = qt // 2
            nc.sync.dma_start(
                out=out[qb * 256:(qb + 1) * 256, h * D:(h + 1) * D].rearrange(
                    "(j p) d -> p j d", p=128
                ),
                in_=osb[:],
            )

    # Pending-PV queue, flowing across head boundaries.
    pvq = []
    pv_state = {}

    def pop_pv():
        h2, qt2, pT2 = pvq.pop(0)
        st = pv_state.setdefault(h2, {})
        if qt2 % 2 == 0:
            st["osb"] = opool.tile([128, 2, D], f32, tag="osb", name="osb")
        emit_pv(h2, qt2, pT2, st["osb"])

    LAG = 4

    def region_of(c):
        kt = 0
        while ROFF[kt + 1] <= c:
            kt += 1
        return kt

    for h in range(QH):
        pT = ppool.tile([128, PCOLS], bf16, tag="pT")
        next_qt = 0  # next query tile to mark PV-ready
        for bi, (b0, bsz, which) in enumerate(BUFS):
            ev = QPREP_EVENTS.get((h, bi))
            if ev is not None:
                qprep_chunk(*ev)
            # drain PV backlog first; drain harder in the last head so the
            # post-loop tail is short
            while len(pvq) > (LAG if h < QH - 1 else 2):
                pop_pv()
            sbuf_tile = sA if which == 0 else sB
            # exact-causal QK chunks packed into this psum buffer
            c = b0
            while c < b0 + bsz:
                kt = region_of(c)
                qoff = kt * 128 + (c - ROFF[kt])  # query index of col c
                step = min(
                    512 - (c - b0) % 512,  # psum bank grid
                    ROFF[kt + 1] - c,      # region end
                    b0 + bsz - c,          # buffer end
                )
                if h == 0:
                    need_k(kt)
                    need_q0(qoff + step)
                nc.tensor.matmul(
                    sbuf_tile[:, c - b0:c - b0 + step],
                    lhsT=kT[:, kt * 128:(kt + 1) * 128],
                    rhs=qT[h][:, qoff:qoff + step],
                    start=True,
                    stop=True,
                )
                c += step
            # one wide exp for the whole buffer
            nc.scalar.activation(
                pT[:, b0:b0 + bsz], sbuf_tile[:, 0:bsz], Exp, scale=SCALE
            )
            # mask any diagonal block this buffer completed (on GpSimd)
            kt = region_of(b0)
            while kt < NT and ROFF[kt] + 128 <= b0 + bsz:
                if ROFF[kt] + 128 > b0:
                    nc.gpsimd.tensor_mul(
                        pT[:, ROFF[kt]:ROFF[kt] + 128],
                        pT[:, ROFF[kt]:ROFF[kt] + 128],
                        keep[:],
                    )
                kt += 1
            # queue query tiles whose last dependency (diag block) is done
            while next_qt < NT and ROFF[next_qt] + 128 <= b0 + bsz:
                pvq.append((h, next_qt, pT))
                next_qt += 1
    while pvq:
        pop_pv()


def _build():
    import concourse.mybir as mybir
    import concourse.tile as tile
    from concourse import bacc
    from contextlib import ExitStack

    nc = bacc.Bacc()
    q = nc.declare_dram_parameter("q", [SEQ, QH * D], mybir.dt.float32, isOutput=False)
    k = nc.declare_dram_parameter("k", [SEQ, D], mybir.dt.float32, isOutput=False)
    v = nc.declare_dram_parameter("v", [SEQ, D], mybir.dt.float32, isOutput=False)
    out = nc.declare_dram_parameter("out", [SEQ, QH * D], mybir.dt.float32, isOutput=True)

    with tile.TileContext(nc) as tc:
        with ExitStack() as ctx:
            _emit(ctx, tc, q[:], k[:], v[:], out[:])
    nc.compile()
    return nc


def _get_nc():
    global _NC
    if _NC is None:
        _NC = _build()
    return _NC


def _ensure_ntff_hook():
    """The agent image's antenv lacks axon_hooks; shim it so trace=True works."""
    import sys
    import types

    if "antenv.axon_hooks" in sys.modules:
        return
    try:
        import antenv
        from trn_agent_boot.trn_boot import _ntff_profile_via_ctypes
    except ImportError:
        return
    mod = types.ModuleType("antenv.axon_hooks")
    hook = [None]
    mod.set_axon_ntff_profile_hook = lambda h: hook.__setitem__(0, h)
    mod.get_axon_ntff_profile_hook = lambda: hook[0]
    sys.modules["antenv.axon_hooks"] = mod
    antenv.axon_hooks = mod
    mod.set_axon_ntff_profile_hook(_ntff_profile_via_ctypes("/opt/axon/libaxon_pjrt.so"))


def _run(q, k, v, trace=False):
    from concourse.bass_utils import run_bass_kernel_spmd

    if trace:
        _ensure_ntff_hook()
    nc = _get_nc()
    in_maps = []
    for i in range(N_CORES):
        in_maps.append(
            {
                "q": np.ascontiguousarray(q[:, i * QH * D:(i + 1) * QH * D]).astype(np.float32, copy=False),
                "k": np.ascontiguousarray(k[:, i * D:(i + 1) * D]).astype(np.float32, copy=False),
                "v": np.ascontiguousarray(v[:, i * D:(i + 1) * D]).astype(np.float32, copy=False),
            }
        )
    res = run_bass_kernel_spmd(nc, in_maps, core_ids=list(range(N_CORES)), trace=trace)
    full = np.concatenate([res.results[i]["out"] for i in range(N_CORES)], axis=1)
    return full.astype(np.float32, copy=False), res


def kernel(q, k, v):
    out, _ = _run(q, k, v, trace=False)
    return out


# revision 36
# speedup vs baseline: 1.2124x; 1.2124x over previous
"""GQA causal attention (S=2048, H=32, KVH=8, D=128) on 8 TRN2 NeuronCores.

Sharding: tensor-parallel over heads. Core i computes query heads
[4i, 4i+4) against KV head i (GQA group size 32/8 = 4). No collectives:
the host slices the inputs per core and concatenates the outputs.

Per-core algorithm (seq=2048, d=128, 4 q-heads, 1 kv-head, causal):
  - Q is loaded whole (all 4 heads) in contiguous 2KB-row DMA chunks --
    4x fewer/larger descriptors than per-head strided loads. K/V load in
    512B-row chunks on the other HWDGE ring. Heads are cast to bf16 on
    the DVE and transposed to [d=128, seq] by PE identity matmuls,
    spread through the previous head's compute.
  - Per head, exact-causal score tiles S^T[kt] = K_tile^T @ Q^T (only
    q >= kt*128) are written PACKED into PSUM buffers B[128,1024] /
    A[128,2048] (B first, so each head's first ACTIVATE has a short
    dependency); ONE wide ACTIVATE(Exp, scale) per buffer writes the
    packed P^T row [128, 17408] bf16 (scores are O(1), so no max
    subtraction). 44 activations instead of 96 -- the scalar engine is
    the steady-state bottleneck at (cols + 352)/1.2GHz per activation.
  - The diagonal 128-col block of each key-tile region is masked by a
    0/1 upper-triangular multiply on the (otherwise idle) GpSimd engine.
  - PV: for each query tile qt, acc[qt] = sum_k2 (P^T slice).T @ [V | 1]
    accumulated in PSUM (3 rotating slices so the DVE normalize never
    stalls the chain); column 128 is the softmax denominator. DVE
    reciprocal + tensor_scalar_mul normalizes; one DMA per 256 rows
    stores the result. PV lags the QK/exp pipeline by a few query tiles
    and flows across head boundaries.
  - One dummy 129-col matmul per buffer cycle parks in a spare PSUM slot
    purely to keep the HAM clock-gate from re-throttling the PE to
    1.2 GHz during scalar-bound stretches.
"""

import numpy as np

SEQ = 2048
D = 128
QH = 4  # query heads per core
N_CORES = 8
SCALE = 0.08838834764831845  # 1/sqrt(128)
NT = SEQ // 128  # 16 tiles of 128 along seq

_NC = None

# packed score-column layout (identical per head)
ROFF = [0]
for _kt in range(1, NT + 1):
    ROFF.append(ROFF[-1] + (SEQ - 128 * (_kt - 1)))
PCOLS = ROFF[NT]  # 17408

# psum buffers: B(1024) first so the head's first ACTIVATE has a short
# dependency chain, then alternate with A(2048); the tail is one extra A.
_SIZES = [1024, 2048] * 5 + [2048]  # sums to PCOLS
BUFS = []
_c = 0
for _sz in _SIZES:
    BUFS.append((_c, _sz, 1 if _sz == 1024 else 0))  # (start, size, pool: 0=A,1=B)
    _c += _sz
assert _c == PCOLS


def _emit(ctx, tc, q, k, v, out):
    import concourse.mybir as mybir
    from concourse import masks

    nc = tc.nc
    f32 = mybir.dt.float32
    bf16 = mybir.dt.bfloat16
    Exp = mybir.ActivationFunctionType.Exp

    singles = ctx.enter_context(tc.tile_pool(name="singles", bufs=1))
    ppool = ctx.enter_context(tc.tile_pool(name="ppool", bufs=2))
    opool = ctx.enter_context(tc.tile_pool(name="opool", bufs=3))
    qbfp = ctx.enter_context(tc.tile_pool(name="qbfp", bufs=2))
    # PSUM budget (8 banks = 16KB/partition):
    #   A 2048 f32 = 4 banks, B 1024 f32 = 2 banks,
    #   PV acc [128,3,129] f32 = 1 bank, transpose+warm staging = 1 bank
    psum_a = ctx.enter_context(tc.tile_pool(name="psum_a", bufs=1, space="PSUM"))
    psum_b = ctx.enter_context(tc.tile_pool(name="psum_b", bufs=1, space="PSUM"))
    psum_o = ctx.enter_context(tc.tile_pool(name="psum_o", bufs=1, space="PSUM"))
    psum_t = ctx.enter_context(tc.tile_pool(name="psum_t", bufs=1, space="PSUM"))

    sA = psum_a.tile([128, 2048], f32, tag="A")
    sB = psum_b.tile([128, 1024], f32, tag="B")
    ops_tri = psum_o.tile([128, 3, D + 1], f32, tag="o")
    # two transpose staging slots inside one PSUM bank (slices rotate)
    tps = psum_t.tile([128, 2, 128], bf16, tag="tp")

    # ---- PE warmup: HAM needs ~3.4us of continuous matmul activity to
    # lift the clock gate to 2.4 GHz; identity transposes don't count.
    warm_src = singles.tile([128, 512], bf16, tag="warm_src")
    nc.vector.memset(warm_src[:], 0.0)

    def warm(n):
        # dummies park in PV slot 0; prep warms are ordered against any
        # overlapping PV chains by the tile framework (correctness-safe)
        for _ in range(n):
            nc.tensor.matmul(
                ops_tri[:, 0, :], lhsT=warm_src[:, 0:128],
                rhs=warm_src[:, 0:D + 1], start=True, stop=True,
            )

    warm(28)

    ident = singles.tile([128, 128], bf16)
    masks.make_identity(nc, ident[:])
    keep = singles.tile([128, 128], bf16)
    masks.make_upper_triangular(nc, keep[:], val=1.0, diag=True)

    # ---- loads: per-head Q chunks just-in-time on the scalar HWDGE ring
    # (each HWDGE engine drives one ~125GB/s queue — loads must be spread
    # over the kernel, not front-loaded); K/V chunks on the sync ring.
    qnatp = ctx.enter_context(tc.tile_pool(name="qnatp", bufs=3))
    qnat = [None] * QH
    _qld_done = set()

    def qld(h, c):
        if (h, c) in _qld_done:
            return
        _qld_done.add((h, c))
        if qnat[h] is None:
            qnat[h] = qnatp.tile([128, NT, D], f32, tag="qnat", name="qnat")
        cs = slice(c * 4, (c + 1) * 4)
        qhr = q[:, h * D:(h + 1) * D].rearrange("(t p) d -> p t d", p=128)
        nc.scalar.dma_start(out=qnat[h][:, cs, :], in_=qhr[:, cs, :])

    kT = singles.tile([128, SEQ], bf16, tag="kT")
    knat = singles.tile([128, NT, 128], f32, tag="knat")
    knat_bf = singles.tile([128, NT, 128], bf16, tag="knat_bf")
    kr = k.rearrange("(t p) d -> p t d", p=128)
    vp = singles.tile([128, NT, D + 1], bf16)
    vnat = singles.tile([128, NT, 128], f32, tag="vnat")
    vr = v.rearrange("(t p) d -> p t d", p=128)
    nc.vector.memset(vp[:, :, D:D + 1], 1.0)

    qT = [
        singles.tile([128, SEQ], bf16, tag=f"qT{h}", name=f"qT{h}")
        for h in range(QH)
    ]

    def qprep_chunk(h, c):
        """Cast + PE-transpose one 4-tile chunk of head h's Q (load must
        already have been issued via qld)."""
        cs = slice(c * 4, (c + 1) * 4)
        qbf = qbfp.tile([128, 4, 128], bf16, tag="qbf", name="qbf")
        nc.vector.tensor_copy(qbf[:], qnat[h][:, cs, :])
        for t in range(4):
            pst = tps[:, t % 2, :]
            nc.tensor.transpose(pst, qbf[:, t, :], ident[:])
            nc.vector.tensor_copy(
                qT[h][:, (c * 4 + t) * 128:(c * 4 + t + 1) * 128], pst
            )
            if t % 2 == 1:
                warm(1)
        if c == 3:
            qnat[h] = None  # release the fp32 staging tile slot

    def kchunk(c):
        cs = slice(c * 4, (c + 1) * 4)
        nc.sync.dma_start(out=knat[:, cs, :], in_=kr[:, cs, :])
        nc.vector.tensor_copy(knat_bf[:, cs, :], knat[:, cs, :])
        for t in range(c * 4, (c + 1) * 4):
            pst = tps[:, t % 2, :]
            nc.tensor.transpose(pst, knat_bf[:, t, :], ident[:])
            nc.vector.tensor_copy(kT[:, t * 128:(t + 1) * 128], pst)
            if t % 2 == 1:
                warm(1)

    def vchunk(c):
        cs = slice(c * 8, (c + 1) * 8)
        nc.sync.dma_start(out=vnat[:, cs, :], in_=vr[:, cs, :])
        nc.vector.tensor_copy(vp[:, cs, 0:D], vnat[:, cs, :])

    # Lazy head-0 prep, emitted just-in-time from inside the buffer walk
    prep_state = {"k": 0, "q0": 0}

    def need_k(kt):
        while prep_state["k"] * 4 <= kt:
            c = prep_state["k"]
            kchunk(c)
            warm(1)
            if c < 2:
                vchunk(c)
            prep_state["k"] += 1

    def need_q0(qhi):
        while prep_state["q0"] * 512 < qhi:
            c = prep_state["q0"]
            qld(0, c)
            if c + 1 < 4:
                qld(0, c + 1)  # stay one load ahead of the transposes
            qprep_chunk(0, c)
            warm(1)
            prep_state["q0"] += 1

    # spread the next head's Q loads and transposes over the current
    # head's buffers (loads lead their transposes by ~2 buffers)
    QPREP_EVENTS = {}
    for _h in range(1, QH):
        QPREP_EVENTS[(_h - 1, 1)] = lambda h=_h: qld(h, 0)
        QPREP_EVENTS[(_h - 1, 2)] = lambda h=_h: qld(h, 1)
        QPREP_EVENTS[(_h - 1, 3)] = lambda h=_h: (qprep_chunk(h, 0), qld(h, 2))
        QPREP_EVENTS[(_h - 1, 5)] = lambda h=_h: (qprep_chunk(h, 1), qld(h, 3))
        QPREP_EVENTS[(_h - 1, 7)] = lambda h=_h: qprep_chunk(h, 2)
        QPREP_EVENTS[(_h - 1, 9)] = lambda h=_h: qprep_chunk(h, 3)

    def emit_pv(h, qt, pT, osb):
        """O[qt] = sum_k2 (P^T slice).T @ [V | 1], then normalize + store."""
        ops = ops_tri[:, qt % 3, :]
        for k2 in range(qt + 1):
            c0 = ROFF[k2] + (qt - k2) * 128
            nc.tensor.matmul(
                ops,
                lhsT=pT[:, c0:c0 + 128],
                rhs=vp[:, k2, :],
                start=(k2 == 0),
                stop=(k2 == qt),
            )
        rec = opool.tile([128, 1], f32, tag="rec")
        nc.vector.reciprocal(rec[:], ops[:, D:D + 1])
        nc.vector.tensor_scalar_mul(osb[:, qt % 2, :], ops[:, 0:D], rec[:])
        if qt % 2 == 1:
            qb = qt // 2
            nc.sync.dma_start(
                out=out[qb * 256:(qb + 1) * 256, h * D:(h + 1) * D].rearrange(
                    "(j p) d -> p j d", p=128
                ),
                in_=osb[:],
            )

    # Pending-PV queue, flowing across head boundaries.
    pvq = []
    pv_state = {}

    def pop_pv():
        h2, qt2, pT2 = pvq.pop(0)
        st = pv_state.setdefault(h2, {})
        if qt2 % 2 == 0:
            st["osb"] = opool.tile([128, 2, D], f32, tag="osb", name="osb")
        emit_pv(h2, qt2, pT2, st["osb"])

    LAG = 4

    def region_of(c):
        kt = 0
        while ROFF[kt + 1] <= c:
            kt += 1
        return kt

    for h in range(QH):
        pT = ppool.tile([128, PCOLS], bf16, tag="pT")
        next_qt = 0  # next query tile to mark PV-ready
        for bi, (b0, bsz, which) in enumerate(BUFS):
            ev = QPREP_EVENTS.get((h, bi))
            if ev is not None:
                ev()
            # drain PV backlog first; drain harder in the last head so the
            # post-loop tail is short
            while len(pvq) > (LAG if h < QH - 1 else 2):
                pop_pv()
            sbuf_tile = sA if which == 0 else sB
            # exact-causal QK chunks packed into this psum buffer
            c = b0
            while c < b0 + bsz:
                kt = region_of(c)
                qoff = kt * 128 + (c - ROFF[kt])  # query index of col c
                step = min(
                    512 - (c - b0) % 512,  # psum bank grid
                    ROFF[kt + 1] - c,      # region end
                    b0 + bsz - c,          # buffer end
                )
                if h == 0:
                    need_k(kt)
                    need_q0(qoff + step)
                nc.tensor.matmul(
                    sbuf_tile[:, c - b0:c - b0 + step],
                    lhsT=kT[:, kt * 128:(kt + 1) * 128],
                    rhs=qT[h][:, qoff:qoff + step],
                    start=True,
                    stop=True,
                )
                c += step
            # one wide exp for the whole buffer
            nc.scalar.activation(
                pT[:, b0:b0 + bsz], sbuf_tile[:, 0:bsz], Exp, scale=SCALE
            )
            # mask any diagonal block this buffer completed (on GpSimd)
            kt = region_of(b0)
            while kt < NT and ROFF[kt] + 128 <= b0 + bsz:
                if ROFF[kt] + 128 > b0:
                    nc.gpsimd.tensor_mul(
                        pT[:, ROFF[kt]:ROFF[kt] + 128],
                        pT[:, ROFF[kt]:ROFF[kt] + 128],
                        keep[:],
                    )
                kt += 1
            # queue query tiles whose last dependency (diag block) is done
            while next_qt < NT and ROFF[next_qt] + 128 <= b0 + bsz:
                pvq.append((h, next_qt, pT))
                next_qt += 1
    while pvq:
        pop_pv()


def _build():
    import concourse.mybir as mybir
    import concourse.tile as tile
    from concourse import bacc
    from contextlib import ExitStack

    nc = bacc.Bacc()
    q = nc.declare_dram_parameter("q", [SEQ, QH * D], mybir.dt.float32, isOutput=False)
    k = nc.declare_dram_parameter("k", [SEQ, D], mybir.dt.float32, isOutput=False)
    v = nc.declare_dram_parameter("v", [SEQ, D], mybir.dt.float32, isOutput=False)
    out = nc.declare_dram_parameter("out", [SEQ, QH * D], mybir.dt.float32, isOutput=True)

    with tile.TileContext(nc) as tc:
        with ExitStack() as ctx:
            _emit(ctx, tc, q[:], k[:], v[:], out[:])
    nc.compile()
    return nc


def _get_nc():
    global _NC
    if _NC is None:
        _NC = _build()
    return _NC


def _ensure_ntff_hook():
    """The agent image's antenv lacks axon_hooks; shim it so trace=True works."""
    import sys
    import types

    if "antenv.axon_hooks" in sys.modules:
        return
    try:
        import antenv
        from trn_agent_boot.trn_boot import _ntff_profile_via_ctypes
    except ImportError:
        return
    mod = types.ModuleType("antenv.axon_hooks")
    hook = [None]
    mod.set_axon_ntff_profile_hook = lambda h: hook.__setitem__(0, h)
    mod.get_axon_ntff_profile_hook = lambda: hook[0]
    sys.modules["antenv.axon_hooks"] = mod
    antenv.axon_hooks = mod
    mod.set_axon_ntff_profile_hook(_ntff_profile_via_ctypes("/opt/axon/libaxon_pjrt.so"))


def _run(q, k, v, trace=False):
    from concourse.bass_utils import run_bass_kernel_spmd

    if trace:
        _ensure_ntff_hook()
    nc = _get_nc()
    in_maps = []
    for i in range(N_CORES):
        in_maps.append(
            {
                "q": np.ascontiguousarray(q[:, i * QH * D:(i + 1) * QH * D]).astype(np.float32, copy=False),
                "k": np.ascontiguousarray(k[:, i * D:(i + 1) * D]).astype(np.float32, copy=False),
                "v": np.ascontiguousarray(v[:, i * D:(i + 1) * D]).astype(np.float32, copy=False),
            }
        )
    res = run_bass_kernel_spmd(nc, in_maps, core_ids=list(range(N_CORES)), trace=trace)
    full = np.concatenate([res.results[i]["out"] for i in range(N_CORES)], axis=1)
    return full.astype(np.float32, copy=False), res


def kernel(q, k, v):
    out, _ = _run(q, k, v, trace=False)
    return out


# revision 42
# speedup vs baseline: 1.2230x; 1.0087x over previous
"""GQA causal attention (S=2048, H=32, KVH=8, D=128) on 8 TRN2 NeuronCores.

Sharding: tensor-parallel over heads. Core i computes query heads
[4i, 4i+4) against KV head i (GQA group size 32/8 = 4). No collectives:
the host slices the inputs per core and concatenates the outputs.

Per-core algorithm (seq=2048, d=128, 4 q-heads, 1 kv-head, causal):
  - Q is loaded whole (all 4 heads) in contiguous 2KB-row DMA chunks --
    4x fewer/larger descriptors than per-head strided loads. K/V load in
    512B-row chunks on the other HWDGE ring. Heads are cast to bf16 on
    the DVE and transposed to [d=128, seq] by PE identity matmuls,
    spread through the previous head's compute.
  - Per head, exact-causal score tiles S^T[kt] = K_tile^T @ Q^T (only
    q >= kt*128) are written PACKED into PSUM buffers B[128,1024] /
    A[128,2048] (B first, so each head's first ACTIVATE has a short
    dependency); ONE wide ACTIVATE(Exp, scale) per buffer writes the
    packed P^T row [128, 17408] bf16 (scores are O(1), so no max
    subtraction). 44 activations instead of 96 -- the scalar engine is
    the steady-state bottleneck at (cols + 352)/1.2GHz per activation.
  - The diagonal 128-col block of each key-tile region is masked by a
    0/1 upper-triangular multiply on the (otherwise idle) GpSimd engine.
  - PV: for each query tile qt, acc[qt] = sum_k2 (P^T slice).T @ [V | 1]
    accumulated in PSUM (3 rotating slices so the DVE normalize never
    stalls the chain); column 128 is the softmax denominator. DVE
    reciprocal + tensor_scalar_mul normalizes; one DMA per 256 rows
    stores the result. PV lags the QK/exp pipeline by a few query tiles
    and flows across head boundaries.
  - One dummy 129-col matmul per buffer cycle parks in a spare PSUM slot
    purely to keep the HAM clock-gate from re-throttling the PE to
    1.2 GHz during scalar-bound stretches.
"""

import numpy as np

SEQ = 2048
D = 128
QH = 4  # query heads per core
N_CORES = 8
SCALE = 0.08838834764831845  # 1/sqrt(128)
NT = SEQ // 128  # 16 tiles of 128 along seq

_NC = None

# packed score-column layout (identical per head)
ROFF = [0]
for _kt in range(1, NT + 1):
    ROFF.append(ROFF[-1] + (SEQ - 128 * (_kt - 1)))
PCOLS = ROFF[NT]  # 17408

# psum buffers: B(1024) first so the head's first ACTIVATE has a short
# dependency chain, then alternate with A(2048); the tail is one extra A.
_SIZES = [1024, 2048] * 5 + [2048]  # sums to PCOLS
BUFS = []
_c = 0
for _sz in _SIZES:
    BUFS.append((_c, _sz, 1 if _sz == 1024 else 0))  # (start, size, pool: 0=A,1=B)
    _c += _sz
assert _c == PCOLS


def _emit(ctx, tc, q, k, v, out):
    import concourse.mybir as mybir
    from concourse import masks

    nc = tc.nc
    f32 = mybir.dt.float32
    bf16 = mybir.dt.bfloat16
    Exp = mybir.ActivationFunctionType.Exp

    singles = ctx.enter_context(tc.tile_pool(name="singles", bufs=1))
    ppool = ctx.enter_context(tc.tile_pool(name="ppool", bufs=2))
    opool = ctx.enter_context(tc.tile_pool(name="opool", bufs=3))
    qbfp = ctx.enter_context(tc.tile_pool(name="qbfp", bufs=2))
    # PSUM budget (8 banks = 16KB/partition):
    #   A 2048 f32 = 4 banks, B 1024 f32 = 2 banks,
    #   PV acc [128,3,129] f32 = 1 bank, transpose+warm staging = 1 bank
    psum_a = ctx.enter_context(tc.tile_pool(name="psum_a", bufs=1, space="PSUM"))
    psum_b = ctx.enter_context(tc.tile_pool(name="psum_b", bufs=1, space="PSUM"))
    psum_o = ctx.enter_context(tc.tile_pool(name="psum_o", bufs=1, space="PSUM"))
    psum_t = ctx.enter_context(tc.tile_pool(name="psum_t", bufs=1, space="PSUM"))

    sA = psum_a.tile([128, 2048], f32, tag="A")
    sB = psum_b.tile([128, 1024], f32, tag="B")
    ops_tri = psum_o.tile([128, 3, D + 1], f32, tag="o")
    # two transpose staging slots inside one PSUM bank (slices rotate)
    tps = psum_t.tile([128, 2, 128], bf16, tag="tp")

    # ---- PE warmup: HAM needs ~3.4us of continuous matmul activity to
    # lift the clock gate to 2.4 GHz; identity transposes don't count.
    warm_src = singles.tile([128, 512], bf16, tag="warm_src")
    nc.vector.memset(warm_src[:], 0.0)

    def warm(n):
        # dummies park in PV slot 0; prep warms are ordered against any
        # overlapping PV chains by the tile framework (correctness-safe)
        for _ in range(n):
            nc.tensor.matmul(
                ops_tri[:, 0, :], lhsT=warm_src[:, 0:128],
                rhs=warm_src[:, 0:D + 1], start=True, stop=True,
            )

    warm(28)

    ident = singles.tile([128, 128], bf16)
    masks.make_identity(nc, ident[:])
    keep = singles.tile([128, 128], bf16)
    masks.make_upper_triangular(nc, keep[:], val=1.0, diag=True)

    # ---- loads: per-head Q chunks just-in-time on the scalar HWDGE ring
    # (each HWDGE engine drives one ~125GB/s queue — loads must be spread
    # over the kernel, not front-loaded); K/V chunks on the sync ring.
    qnatp = ctx.enter_context(tc.tile_pool(name="qnatp", bufs=3))
    qnat = [None] * QH
    _qld_done = set()

    def qld(h, c):
        if (h, c) in _qld_done:
            return
        _qld_done.add((h, c))
        if qnat[h] is None:
            qnat[h] = qnatp.tile([128, NT, D], f32, tag="qnat", name="qnat")
        cs = slice(c * 4, (c + 1) * 4)
        qhr = q[:, h * D:(h + 1) * D].rearrange("(t p) d -> p t d", p=128)
        nc.scalar.dma_start(out=qnat[h][:, cs, :], in_=qhr[:, cs, :])

    kT = singles.tile([128, SEQ], bf16, tag="kT")
    knat = singles.tile([128, NT, 128], f32, tag="knat")
    knat_bf = singles.tile([128, NT, 128], bf16, tag="knat_bf")
    kr = k.rearrange("(t p) d -> p t d", p=128)
    vp = singles.tile([128, NT, D + 1], bf16)
    vnat = singles.tile([128, NT, 128], f32, tag="vnat")
    vr = v.rearrange("(t p) d -> p t d", p=128)
    nc.vector.memset(vp[:, :, D:D + 1], 1.0)

    qT = [
        singles.tile([128, SEQ], bf16, tag=f"qT{h}", name=f"qT{h}")
        for h in range(QH)
    ]

    # heads 2-3: Q^T via background DMA engines (SWDGE fp32->bf16 cast to
    # a DRAM scratch, then XBAR-transpose into SBUF), staggered into idle
    # fabric windows so they never compete with critical loads.
    q_sc = {h: nc.dram_tensor(f"q_sc{h}", [SEQ, D], bf16) for h in (2, 3)}

    def qcast(h):
        nc.gpsimd.dma_start(out=q_sc[h][:, :], in_=q[:, h * D:(h + 1) * D])

    def qtrans(h):
        nc.sync.dma_start(out=qT[h][:, :], in_=q_sc[h][:, :], transpose=True)

    def qprep_chunk(h, c):
        """Cast + PE-transpose one 4-tile chunk of head h's Q (load must
        already have been issued via qld)."""
        cs = slice(c * 4, (c + 1) * 4)
        qbf = qbfp.tile([128, 4, 128], bf16, tag="qbf", name="qbf")
        nc.vector.tensor_copy(qbf[:], qnat[h][:, cs, :])
        for t in range(4):
            pst = tps[:, t % 2, :]
            nc.tensor.transpose(pst, qbf[:, t, :], ident[:])
            nc.vector.tensor_copy(
                qT[h][:, (c * 4 + t) * 128:(c * 4 + t + 1) * 128], pst
            )
        if c == 3:
            qnat[h] = None  # release the fp32 staging tile slot

    def kchunk(c):
        cs = slice(c * 4, (c + 1) * 4)
        nc.sync.dma_start(out=knat[:, cs, :], in_=kr[:, cs, :])
        nc.vector.tensor_copy(knat_bf[:, cs, :], knat[:, cs, :])
        for t in range(c * 4, (c + 1) * 4):
            pst = tps[:, t % 2, :]
            nc.tensor.transpose(pst, knat_bf[:, t, :], ident[:])
            nc.vector.tensor_copy(kT[:, t * 128:(t + 1) * 128], pst)

    def vchunk(c):
        cs = slice(c * 8, (c + 1) * 8)
        nc.sync.dma_start(out=vnat[:, cs, :], in_=vr[:, cs, :])
        nc.vector.tensor_copy(vp[:, cs, 0:D], vnat[:, cs, :])

    # Lazy head-0 prep, emitted just-in-time from inside the buffer walk
    prep_state = {"k": 0, "q0": 0}

    def need_k(kt):
        while prep_state["k"] * 4 <= kt:
            c = prep_state["k"]
            kchunk(c)
            warm(1)
            if c < 2:
                vchunk(c)
            prep_state["k"] += 1

    def need_q0(qhi):
        while prep_state["q0"] * 512 < qhi:
            c = prep_state["q0"]
            qld(0, c)
            if c + 1 < 4:
                qld(0, c + 1)  # stay one load ahead of the transposes
            qprep_chunk(0, c)
            warm(1)
            prep_state["q0"] += 1

    # head 1 Q via PE transposes spread over head 0; heads 2-3 via the
    # background DMA path in fabric-idle windows
    QPREP_EVENTS = {
        (0, 1): lambda: qld(1, 0),
        (0, 2): lambda: qld(1, 1),
        (0, 3): lambda: (qprep_chunk(1, 0), qld(1, 2)),
        (0, 5): lambda: (qprep_chunk(1, 1), qld(1, 3)),
        (0, 7): lambda: qprep_chunk(1, 2),
        (0, 8): lambda: qcast(2),
        (0, 9): lambda: qprep_chunk(1, 3),
        (1, 5): lambda: qcast(3),
        (1, 8): lambda: qtrans(2),
        (2, 8): lambda: qtrans(3),
    }

    def emit_pv(h, qt, pT, osb):
        """O[qt] = sum_k2 (P^T slice).T @ [V | 1], then normalize + store."""
        ops = ops_tri[:, qt % 3, :]
        for k2 in range(qt + 1):
            c0 = ROFF[k2] + (qt - k2) * 128
            nc.tensor.matmul(
                ops,
                lhsT=pT[:, c0:c0 + 128],
                rhs=vp[:, k2, :],
                start=(k2 == 0),
                stop=(k2 == qt),
            )
        rec = opool.tile([128, 1], f32, tag="rec")
        nc.vector.reciprocal(rec[:], ops[:, D:D + 1])
        nc.vector.tensor_scalar_mul(osb[:, qt % 2, :], ops[:, 0:D], rec[:])
        if qt % 2 == 1:
            qb = qt // 2
            nc.sync.dma_start(
                out=out[qb * 256:(qb + 1) * 256, h * D:(h + 1) * D].rearrange(
                    "(j p) d -> p j d", p=128
                ),
                in_=osb[:],
            )

    # Pending-PV queue, flowing across head boundaries.
    pvq = []
    pv_state = {}

    def pop_pv():
        h2, qt2, pT2 = pvq.pop(0)
        st = pv_state.setdefault(h2, {})
        if qt2 % 2 == 0:
            st["osb"] = opool.tile([128, 2, D], f32, tag="osb", name="osb")
        emit_pv(h2, qt2, pT2, st["osb"])

    # per-head PV lag: 0 during head 0 fills the ramp (and keeps the HAM
    # clock gate engaged); small in the last head to shrink the drain tail
    HLAG = [0, 3, 3, 1]

    def region_of(c):
        kt = 0
        while ROFF[kt + 1] <= c:
            kt += 1
        return kt

    for h in range(QH):
        pT = ppool.tile([128, PCOLS], bf16, tag="pT")
        next_qt = 0  # next query tile to mark PV-ready
        for bi, (b0, bsz, which) in enumerate(BUFS):
            ev = QPREP_EVENTS.get((h, bi))
            if ev is not None:
                ev()
            # drain PV backlog down to the per-head lag
            while len(pvq) > HLAG[h]:
                pop_pv()
            sbuf_tile = sA if which == 0 else sB
            # exact-causal QK chunks packed into this psum buffer
            c = b0
            while c < b0 + bsz:
                kt = region_of(c)
                qoff = kt * 128 + (c - ROFF[kt])  # query index of col c
                step = min(
                    512 - (c - b0) % 512,  # psum bank grid
                    ROFF[kt + 1] - c,      # region end
                    b0 + bsz - c,          # buffer end
                )
                if h == 0:
                    need_k(kt)
                    need_q0(qoff + step)
                nc.tensor.matmul(
                    sbuf_tile[:, c - b0:c - b0 + step],
                    lhsT=kT[:, kt * 128:(kt + 1) * 128],
                    rhs=qT[h][:, qoff:qoff + step],
                    start=True,
                    stop=True,
                )
                c += step
            # one wide exp for the whole buffer
            nc.scalar.activation(
                pT[:, b0:b0 + bsz], sbuf_tile[:, 0:bsz], Exp, scale=SCALE
            )
            # mask any diagonal block this buffer completed (on GpSimd)
            kt = region_of(b0)
            while kt < NT and ROFF[kt] + 128 <= b0 + bsz:
                if ROFF[kt] + 128 > b0:
                    nc.gpsimd.tensor_mul(
                        pT[:, ROFF[kt]:ROFF[kt] + 128],
                        pT[:, ROFF[kt]:ROFF[kt] + 128],
                        keep[:],
                    )
                kt += 1
            # queue query tiles whose last dependency (diag block) is done
            while next_qt < NT and ROFF[next_qt] + 128 <= b0 + bsz:
                pvq.append((h, next_qt, pT))
                next_qt += 1
    while pvq:
        pop_pv()


def _build():
    import concourse.mybir as mybir
    import concourse.tile as tile
    from concourse import bacc
    from contextlib import ExitStack

    nc = bacc.Bacc()
    q = nc.declare_dram_parameter("q", [SEQ, QH * D], mybir.dt.float32, isOutput=False)
    k = nc.declare_dram_parameter("k", [SEQ, D], mybir.dt.float32, isOutput=False)
    v = nc.declare_dram_parameter("v", [SEQ, D], mybir.dt.float32, isOutput=False)
    out = nc.declare_dram_parameter("out", [SEQ, QH * D], mybir.dt.float32, isOutput=True)

    with tile.TileContext(nc) as tc:
        with ExitStack() as ctx:
            _emit(ctx, tc, q[:], k[:], v[:], out[:])
    nc.compile()
    return nc


def _get_nc():
    global _NC
    if _NC is None:
        _NC = _build()
    return _NC


def _ensure_ntff_hook():
    """The agent image's antenv lacks axon_hooks; shim it so trace=True works."""
    import sys
    import types

    if "antenv.axon_hooks" in sys.modules:
        return
    try:
        import antenv
        from trn_agent_boot.trn_boot import _ntff_profile_via_ctypes
    except ImportError:
        return
    mod = types.ModuleType("antenv.axon_hooks")
    hook = [None]
    mod.set_axon_ntff_profile_hook = lambda h: hook.__setitem__(0, h)
    mod.get_axon_ntff_profile_hook = lambda: hook[0]
    sys.modules["antenv.axon_hooks"] = mod
    antenv.axon_hooks = mod
    mod.set_axon_ntff_profile_hook(_ntff_profile_via_ctypes("/opt/axon/libaxon_pjrt.so"))


def _run(q, k, v, trace=False):
    from concourse.bass_utils import run_bass_kernel_spmd

    if trace:
        _ensure_ntff_hook()
    nc = _get_nc()
    in_maps = []
    for i in range(N_CORES):
        in_maps.append(
            {
                "q": np.ascontiguousarray(q[:, i * QH * D:(i + 1) * QH * D]).astype(np.float32, copy=False),
                "k": np.ascontiguousarray(k[:, i * D:(i + 1) * D]).astype(np.float32, copy=False),
                "v": np.ascontiguousarray(v[:, i * D:(i + 1) * D]).astype(np.float32, copy=False),
            }
        )
    res = run_bass_kernel_spmd(nc, in_maps, core_ids=list(range(N_CORES)), trace=trace)
    full = np.concatenate([res.results[i]["out"] for i in range(N_CORES)], axis=1)
    return full.astype(np.float32, copy=False), res


def kernel(q, k, v):
    out, _ = _run(q, k, v, trace=False)
    return out


# revision 46
# speedup vs baseline: 1.2631x; 1.0328x over previous
"""GQA causal attention (S=2048, H=32, KVH=8, D=128) on 8 TRN2 NeuronCores.

Sharding: tensor-parallel over heads. Core i computes query heads
[4i, 4i+4) against KV head i (GQA group size 32/8 = 4). No collectives:
the host slices the inputs per core and concatenates the outputs.

Per-core algorithm (seq=2048, d=128, 4 q-heads, 1 kv-head, causal):
  - Q is loaded whole (all 4 heads) in contiguous 2KB-row DMA chunks --
    4x fewer/larger descriptors than per-head strided loads. K/V load in
    512B-row chunks on the other HWDGE ring. Heads are cast to bf16 on
    the DVE and transposed to [d=128, seq] by PE identity matmuls,
    spread through the previous head's compute.
  - Per head, exact-causal score tiles S^T[kt] = K_tile^T @ Q^T (only
    q >= kt*128) are written PACKED into PSUM buffers B[128,1024] /
    A[128,2048] (B first, so each head's first ACTIVATE has a short
    dependency); ONE wide ACTIVATE(Exp, scale) per buffer writes the
    packed P^T row [128, 17408] bf16 (scores are O(1), so no max
    subtraction). 44 activations instead of 96 -- the scalar engine is
    the steady-state bottleneck at (cols + 352)/1.2GHz per activation.
  - The diagonal 128-col block of each key-tile region is masked by a
    0/1 upper-triangular multiply on the (otherwise idle) GpSimd engine.
  - PV: for each query tile qt, acc[qt] = sum_k2 (P^T slice).T @ [V | 1]
    accumulated in PSUM (3 rotating slices so the DVE normalize never
    stalls the chain); column 128 is the softmax denominator. DVE
    reciprocal + tensor_scalar_mul normalizes; one DMA per 256 rows
    stores the result. PV lags the QK/exp pipeline by a few query tiles
    and flows across head boundaries.
  - One dummy 129-col matmul per buffer cycle parks in a spare PSUM slot
    purely to keep the HAM clock-gate from re-throttling the PE to
    1.2 GHz during scalar-bound stretches.
"""

import numpy as np

SEQ = 2048
D = 128
QH = 4  # query heads per core
N_CORES = 8
SCALE = 0.08838834764831845  # 1/sqrt(128)
NT = SEQ // 128  # 16 tiles of 128 along seq

_NC = None

# packed score-column layout (identical per head), QUERY-PAIR-MAJOR:
# row qtb (query tiles 2*qtb, 2*qtb+1) holds regions kt = 0..2*qtb+1 of
# [128 keys x 256 queries] each, except the last (kt = 2*qtb+1) which is
# [128 x 128] (odd query tile only — exact causal, zero wasted exp).
# This ordering needs only a prefix of Q/K per row, so the pipeline
# starts with ~0.3MB of input instead of a whole head's worth.
REGIONS = []   # (start_col, width, kt, qstart) in packing order
RSTART = {}    # (qtb, kt) -> start col
ROWEND = []    # packed col at which row qtb completes
_c = 0
for _qtb in range(NT // 2):
    for _kt in range(2 * _qtb + 2):
        _w = 128 if _kt == 2 * _qtb + 1 else 256
        _qs = 256 * _qtb + (128 if _kt == 2 * _qtb + 1 else 0)
        REGIONS.append((_c, _w, _kt, _qs))
        RSTART[(_qtb, _kt)] = _c
        _c += _w
    ROWEND.append(_c)
PCOLS = _c  # 17408

# psum buffers: B(1024) first so the head's first ACTIVATE has a short
# dependency chain, then alternate with A(2048); the tail is one extra A.
_SIZES = [1024, 2048] * 5 + [2048]  # sums to PCOLS
BUFS = []
_c = 0
for _sz in _SIZES:
    BUFS.append((_c, _sz, 1 if _sz == 1024 else 0))  # (start, size, pool: 0=A,1=B)
    _c += _sz
assert _c == PCOLS


def _emit(ctx, tc, q, k, v, out):
    import concourse.mybir as mybir
    from concourse import masks

    nc = tc.nc
    f32 = mybir.dt.float32
    bf16 = mybir.dt.bfloat16
    Exp = mybir.ActivationFunctionType.Exp

    singles = ctx.enter_context(tc.tile_pool(name="singles", bufs=1))
    ppool = ctx.enter_context(tc.tile_pool(name="ppool", bufs=2))
    opool = ctx.enter_context(tc.tile_pool(name="opool", bufs=3))
    qbfp = ctx.enter_context(tc.tile_pool(name="qbfp", bufs=2))
    # PSUM budget (8 banks = 16KB/partition):
    #   A 2048 f32 = 4 banks, B 1024 f32 = 2 banks,
    #   PV acc [128,3,129] f32 = 1 bank, transpose+warm staging = 1 bank
    psum_a = ctx.enter_context(tc.tile_pool(name="psum_a", bufs=1, space="PSUM"))
    psum_b = ctx.enter_context(tc.tile_pool(name="psum_b", bufs=1, space="PSUM"))
    psum_o = ctx.enter_context(tc.tile_pool(name="psum_o", bufs=1, space="PSUM"))
    psum_t = ctx.enter_context(tc.tile_pool(name="psum_t", bufs=1, space="PSUM"))

    sA = psum_a.tile([128, 2048], f32, tag="A")
    sB = psum_b.tile([128, 1024], f32, tag="B")
    ops_tri = psum_o.tile([128, 3, D + 1], f32, tag="o")
    # two transpose staging slots inside one PSUM bank (slices rotate)
    tps = psum_t.tile([128, 2, 128], bf16, tag="tp")

    # ---- PE warmup: HAM needs ~3.4us of continuous matmul activity to
    # lift the clock gate to 2.4 GHz; identity transposes don't count.
    warm_src = singles.tile([128, 512], bf16, tag="warm_src")
    nc.vector.memset(warm_src[:], 0.0)

    def warm(n):
        # dummies park in PV slot 0; prep warms are ordered against any
        # overlapping PV chains by the tile framework (correctness-safe)
        for _ in range(n):
            nc.tensor.matmul(
                ops_tri[:, 0, :], lhsT=warm_src[:, 0:128],
                rhs=warm_src[:, 0:D + 1], start=True, stop=True,
            )

    warm(28)

    ident = singles.tile([128, 128], bf16)
    masks.make_identity(nc, ident[:])
    keep = singles.tile([128, 128], bf16)
    masks.make_upper_triangular(nc, keep[:], val=1.0, diag=True)

    # ---- loads: per-head Q chunks just-in-time on the scalar HWDGE ring
    # (each HWDGE engine drives one ~125GB/s queue — loads must be spread
    # over the kernel, not front-loaded); K/V chunks on the sync ring.
    qnatp = ctx.enter_context(tc.tile_pool(name="qnatp", bufs=3))
    qnat = [None] * QH
    _qld_done = set()

    def qld(h, c):
        if (h, c) in _qld_done:
            return
        _qld_done.add((h, c))
        if qnat[h] is None:
            qnat[h] = qnatp.tile([128, NT, D], f32, tag="qnat", name="qnat")
        cs = slice(c * 4, (c + 1) * 4)
        qhr = q[:, h * D:(h + 1) * D].rearrange("(t p) d -> p t d", p=128)
        nc.scalar.dma_start(out=qnat[h][:, cs, :], in_=qhr[:, cs, :])

    kT = singles.tile([128, SEQ], bf16, tag="kT")
    knat = singles.tile([128, NT, 128], f32, tag="knat")
    knat_bf = singles.tile([128, NT, 128], bf16, tag="knat_bf")
    kr = k.rearrange("(t p) d -> p t d", p=128)
    vp = singles.tile([128, NT, D + 1], bf16)
    vnat = singles.tile([128, NT, 128], f32, tag="vnat")
    vr = v.rearrange("(t p) d -> p t d", p=128)
    nc.vector.memset(vp[:, :, D:D + 1], 1.0)

    qT = [
        singles.tile([128, SEQ], bf16, tag=f"qT{h}", name=f"qT{h}")
        for h in range(QH)
    ]

    # heads 2-3: Q^T via background DMA engines (SWDGE fp32->bf16 cast to
    # a DRAM scratch, then XBAR-transpose into SBUF), staggered into idle
    # fabric windows so they never compete with critical loads.
    q_sc = {h: nc.dram_tensor(f"q_sc{h}", [SEQ, D], bf16) for h in (2, 3)}

    def qcast(h):
        nc.gpsimd.dma_start(out=q_sc[h][:, :], in_=q[:, h * D:(h + 1) * D])

    def qtrans(h):
        nc.sync.dma_start(out=qT[h][:, :], in_=q_sc[h][:, :], transpose=True)

    def qprep_chunk(h, c):
        """Cast + PE-transpose one 4-tile chunk of head h's Q (load must
        already have been issued via qld)."""
        cs = slice(c * 4, (c + 1) * 4)
        qbf = qbfp.tile([128, 4, 128], bf16, tag="qbf", name="qbf")
        nc.vector.tensor_copy(qbf[:], qnat[h][:, cs, :])
        for t in range(4):
            pst = tps[:, t % 2, :]
            nc.tensor.transpose(pst, qbf[:, t, :], ident[:])
            nc.vector.tensor_copy(
                qT[h][:, (c * 4 + t) * 128:(c * 4 + t + 1) * 128], pst
            )
        if c == 3:
            qnat[h] = None  # release the fp32 staging tile slot

    def kchunk(c):
        cs = slice(c * 4, (c + 1) * 4)
        nc.sync.dma_start(out=knat[:, cs, :], in_=kr[:, cs, :])
        nc.vector.tensor_copy(knat_bf[:, cs, :], knat[:, cs, :])
        for t in range(c * 4, (c + 1) * 4):
            pst = tps[:, t % 2, :]
            nc.tensor.transpose(pst, knat_bf[:, t, :], ident[:])
            nc.vector.tensor_copy(kT[:, t * 128:(t + 1) * 128], pst)

    def vchunk(c):
        cs = slice(c * 8, (c + 1) * 8)
        nc.sync.dma_start(out=vnat[:, cs, :], in_=vr[:, cs, :])
        nc.vector.tensor_copy(vp[:, cs, 0:D], vnat[:, cs, :])

    # Lazy head-0 prep, emitted just-in-time from inside the buffer walk
    prep_state = {"k": 0, "q0": 0}

    def need_k(kt):
        while prep_state["k"] * 4 <= kt:
            c = prep_state["k"]
            kchunk(c)
            warm(1)
            if c < 2:
                vchunk(c)
            prep_state["k"] += 1

    def need_q0(qhi):
        while prep_state["q0"] * 512 < qhi:
            c = prep_state["q0"]
            qld(0, c)
            if c + 1 < 4:
                qld(0, c + 1)  # stay one load ahead of the transposes
            qprep_chunk(0, c)
            warm(1)
            prep_state["q0"] += 1

    # head 1 Q via PE transposes spread over head 0; heads 2-3 via the
    # background DMA path in fabric-idle windows
    QPREP_EVENTS = {
        (0, 1): lambda: qld(1, 0),
        (0, 2): lambda: qld(1, 1),
        (0, 3): lambda: (qprep_chunk(1, 0), qld(1, 2)),
        (0, 5): lambda: (qprep_chunk(1, 1), qld(1, 3)),
        (0, 7): lambda: qprep_chunk(1, 2),
        (0, 8): lambda: qcast(2),
        (0, 9): lambda: qprep_chunk(1, 3),
        (1, 5): lambda: qcast(3),
        (1, 8): lambda: qtrans(2),
        (2, 8): lambda: qtrans(3),
    }

    def emit_pv(h, qt, pT, osb):
        """O[qt] = sum_k2 (P^T slice).T @ [V | 1], then normalize + store."""
        ops = ops_tri[:, qt % 3, :]
        qtb = qt // 2
        for k2 in range(qt + 1):
            c0 = RSTART[(qtb, k2)] + (
                128 if (qt % 2 == 1 and k2 < 2 * qtb + 1) else 0
            )
            nc.tensor.matmul(
                ops,
                lhsT=pT[:, c0:c0 + 128],
                rhs=vp[:, k2, :],
                start=(k2 == 0),
                stop=(k2 == qt),
            )
        rec = opool.tile([128, 1], f32, tag="rec")
        nc.vector.reciprocal(rec[:], ops[:, D:D + 1])
        nc.vector.tensor_scalar_mul(osb[:, qt % 2, :], ops[:, 0:D], rec[:])
        if qt % 2 == 1:
            qb = qt // 2
            nc.sync.dma_start(
                out=out[qb * 256:(qb + 1) * 256, h * D:(h + 1) * D].rearrange(
                    "(j p) d -> p j d", p=128
                ),
                in_=osb[:],
            )

    # Pending-PV queue, flowing across head boundaries.
    pvq = []
    pv_state = {}

    def pop_pv():
        h2, qt2, pT2 = pvq.pop(0)
        st = pv_state.setdefault(h2, {})
        if qt2 % 2 == 0:
            st["osb"] = opool.tile([128, 2, D], f32, tag="osb", name="osb")
        emit_pv(h2, qt2, pT2, st["osb"])

    # per-head PV lag (in query tiles): small during head 0 so PV work
    # flows from the start (keeps the HAM clock gate engaged); small in
    # the last head so the post-loop drain tail is short
    HLAG = [1, 3, 3, 1]

    _ri = {"i": 0}

    def region_of(c):
        # REGIONS is walked strictly left-to-right within each head
        i = _ri["i"]
        while i + 1 < len(REGIONS) and REGIONS[i + 1][0] <= c:
            i += 1
        _ri["i"] = i
        return REGIONS[i]

    for h in range(QH):
        pT = ppool.tile([128, PCOLS], bf16, tag="pT")
        next_row = 0  # next query-pair row to mark PV-ready
        _ri["i"] = 0
        for bi, (b0, bsz, which) in enumerate(BUFS):
            ev = QPREP_EVENTS.get((h, bi))
            if ev is not None:
                ev()
            # drain PV backlog down to the per-head lag
            while len(pvq) > HLAG[h]:
                pop_pv()
            sbuf_tile = sA if which == 0 else sB
            # exact-causal QK chunks packed into this psum buffer
            c = b0
            while c < b0 + bsz:
                r0, rw, kt, qs = region_of(c)
                qoff = qs + (c - r0)  # query index of col c
                step = min(
                    512 - (c - b0) % 512,  # psum bank grid
                    r0 + rw - c,           # region end
                    b0 + bsz - c,          # buffer end
                )
                if h == 0:
                    need_k(kt)
                    need_q0(qoff + step)
                nc.tensor.matmul(
                    sbuf_tile[:, c - b0:c - b0 + step],
                    lhsT=kT[:, kt * 128:(kt + 1) * 128],
                    rhs=qT[h][:, qoff:qoff + step],
                    start=True,
                    stop=True,
                )
                c += step
            # one wide exp for the whole buffer
            nc.scalar.activation(
                pT[:, b0:b0 + bsz], sbuf_tile[:, 0:bsz], Exp, scale=SCALE
            )
            # rows completed by this buffer: mask the two diagonal tiles
            # (on GpSimd), then queue their query tiles for PV
            while next_row < NT // 2 and ROWEND[next_row] <= b0 + bsz:
                qtb = next_row
                for mc in (RSTART[(qtb, 2 * qtb)], RSTART[(qtb, 2 * qtb + 1)]):
                    nc.gpsimd.tensor_mul(
                        pT[:, mc:mc + 128], pT[:, mc:mc + 128], keep[:]
                    )
                pvq.append((h, 2 * qtb, pT))
                pvq.append((h, 2 * qtb + 1, pT))
                next_row += 1
    while pvq:
        pop_pv()


def _build():
    import concourse.mybir as mybir
    import concourse.tile as tile
    from concourse import bacc
    from contextlib import ExitStack

    nc = bacc.Bacc()
    q = nc.declare_dram_parameter("q", [SEQ, QH * D], mybir.dt.float32, isOutput=False)
    k = nc.declare_dram_parameter("k", [SEQ, D], mybir.dt.float32, isOutput=False)
    v = nc.declare_dram_parameter("v", [SEQ, D], mybir.dt.float32, isOutput=False)
    out = nc.declare_dram_parameter("out", [SEQ, QH * D], mybir.dt.float32, isOutput=True)

    with tile.TileContext(nc) as tc:
        with ExitStack() as ctx:
            _emit(ctx, tc, q[:], k[:], v[:], out[:])
    nc.compile()
    return nc


def _get_nc():
    global _NC
    if _NC is None:
        _NC = _build()
    return _NC


def _ensure_ntff_hook():
    """The agent image's antenv lacks axon_hooks; shim it so trace=True works."""
    import sys
    import types

    if "antenv.axon_hooks" in sys.modules:
        return
    try:
        import antenv
        from trn_agent_boot.trn_boot import _ntff_profile_via_ctypes
    except ImportError:
        return
    mod = types.ModuleType("antenv.axon_hooks")
    hook = [None]
    mod.set_axon_ntff_profile_hook = lambda h: hook.__setitem__(0, h)
    mod.get_axon_ntff_profile_hook = lambda: hook[0]
    sys.modules["antenv.axon_hooks"] = mod
    antenv.axon_hooks = mod
    mod.set_axon_ntff_profile_hook(_ntff_profile_via_ctypes("/opt/axon/libaxon_pjrt.so"))


def _run(q, k, v, trace=False):
    from concourse.bass_utils import run_bass_kernel_spmd

    if trace:
        _ensure_ntff_hook()
    nc = _get_nc()
    in_maps = []
    for i in range(N_CORES):
        in_maps.append(
            {
                "q": np.ascontiguousarray(q[:, i * QH * D:(i + 1) * QH * D]).astype(np.float32, copy=False),
                "k": np.ascontiguousarray(k[:, i * D:(i + 1) * D]).astype(np.float32, copy=False),
                "v": np.ascontiguousarray(v[:, i * D:(i + 1) * D]).astype(np.float32, copy=False),
            }
        )
    res = run_bass_kernel_spmd(nc, in_maps, core_ids=list(range(N_CORES)), trace=trace)
    full = np.concatenate([res.results[i]["out"] for i in range(N_CORES)], axis=1)
    return full.astype(np.float32, copy=False), res


def kernel(q, k, v):
    out, _ = _run(q, k, v, trace=False)
    return out


# revision 49
# speedup vs baseline: 1.3127x; 1.0392x over previous
"""GQA causal attention (S=2048, H=32, KVH=8, D=128) on 8 TRN2 NeuronCores.

Sharding: tensor-parallel over heads. Core i computes query heads
[4i, 4i+4) against KV head i (GQA group size 32/8 = 4). No collectives:
the host slices the inputs per core and concatenates the outputs.

Per-core algorithm (seq=2048, d=128, 4 q-heads, 1 kv-head, causal):
  - Q is loaded whole (all 4 heads) in contiguous 2KB-row DMA chunks --
    4x fewer/larger descriptors than per-head strided loads. K/V load in
    512B-row chunks on the other HWDGE ring. Heads are cast to bf16 on
    the DVE and transposed to [d=128, seq] by PE identity matmuls,
    spread through the previous head's compute.
  - Per head, exact-causal score tiles S^T[kt] = K_tile^T @ Q^T (only
    q >= kt*128) are written PACKED into PSUM buffers B[128,1024] /
    A[128,2048] (B first, so each head's first ACTIVATE has a short
    dependency); ONE wide ACTIVATE(Exp, scale) per buffer writes the
    packed P^T row [128, 17408] bf16 (scores are O(1), so no max
    subtraction). 44 activations instead of 96 -- the scalar engine is
    the steady-state bottleneck at (cols + 352)/1.2GHz per activation.
  - The diagonal 128-col block of each key-tile region is masked by a
    0/1 upper-triangular multiply on the (otherwise idle) GpSimd engine.
  - PV: for each query tile qt, acc[qt] = sum_k2 (P^T slice).T @ [V | 1]
    accumulated in PSUM (3 rotating slices so the DVE normalize never
    stalls the chain); column 128 is the softmax denominator. DVE
    reciprocal + tensor_scalar_mul normalizes; one DMA per 256 rows
    stores the result. PV lags the QK/exp pipeline by a few query tiles
    and flows across head boundaries.
  - One dummy 129-col matmul per buffer cycle parks in a spare PSUM slot
    purely to keep the HAM clock-gate from re-throttling the PE to
    1.2 GHz during scalar-bound stretches.
"""

import numpy as np

SEQ = 2048
D = 128
QH = 4  # query heads per core
N_CORES = 8
SCALE = 0.08838834764831845  # 1/sqrt(128)
NT = SEQ // 128  # 16 tiles of 128 along seq

_NC = None

# packed score-column layout (identical per head), QUERY-PAIR-MAJOR:
# row qtb (query tiles 2*qtb, 2*qtb+1) holds regions kt = 0..2*qtb+1 of
# [128 keys x 256 queries] each, except the last (kt = 2*qtb+1) which is
# [128 x 128] (odd query tile only — exact causal, zero wasted exp).
# This ordering needs only a prefix of Q/K per row, so the pipeline
# starts with ~0.3MB of input instead of a whole head's worth.
REGIONS = []   # (start_col, width, kt, qstart) in packing order
RSTART = {}    # (qtb, kt) -> start col
ROWEND = []    # packed col at which row qtb completes
_c = 0
for _qtb in range(NT // 2):
    for _kt in range(2 * _qtb + 2):
        _w = 128 if _kt == 2 * _qtb + 1 else 256
        _qs = 256 * _qtb + (128 if _kt == 2 * _qtb + 1 else 0)
        REGIONS.append((_c, _w, _kt, _qs))
        RSTART[(_qtb, _kt)] = _c
        _c += _w
    ROWEND.append(_c)
PCOLS = _c  # 17408

# psum buffers: B(1024) first so the head's first ACTIVATE has a short
# dependency chain, then alternate with A(2048); the tail is one extra A.
_SIZES = [1024, 2048] * 5 + [2048]  # sums to PCOLS
BUFS = []
_c = 0
for _sz in _SIZES:
    BUFS.append((_c, _sz, 1 if _sz == 1024 else 0))  # (start, size, pool: 0=A,1=B)
    _c += _sz
assert _c == PCOLS


def _emit(ctx, tc, q, k, v, out):
    import concourse.mybir as mybir
    from concourse import masks

    nc = tc.nc
    f32 = mybir.dt.float32
    bf16 = mybir.dt.bfloat16
    Exp = mybir.ActivationFunctionType.Exp

    singles = ctx.enter_context(tc.tile_pool(name="singles", bufs=1))
    ppool = ctx.enter_context(tc.tile_pool(name="ppool", bufs=2))
    opool = ctx.enter_context(tc.tile_pool(name="opool", bufs=3))
    qbfp = ctx.enter_context(tc.tile_pool(name="qbfp", bufs=2))
    # PSUM budget (8 banks = 16KB/partition):
    #   A 2048 f32 = 4 banks, B 1024 f32 = 2 banks,
    #   PV acc [128,3,129] f32 = 1 bank, transpose+warm staging = 1 bank
    psum_a = ctx.enter_context(tc.tile_pool(name="psum_a", bufs=1, space="PSUM"))
    psum_b = ctx.enter_context(tc.tile_pool(name="psum_b", bufs=1, space="PSUM"))
    psum_o = ctx.enter_context(tc.tile_pool(name="psum_o", bufs=1, space="PSUM"))
    psum_t = ctx.enter_context(tc.tile_pool(name="psum_t", bufs=1, space="PSUM"))

    sA = psum_a.tile([128, 2048], f32, tag="A")
    sB = psum_b.tile([128, 1024], f32, tag="B")
    ops_tri = psum_o.tile([128, 3, D + 1], f32, tag="o")
    # two transpose staging slots inside one PSUM bank (slices rotate)
    tps = psum_t.tile([128, 2, 128], bf16, tag="tp")

    # ---- PE warmup: HAM needs ~3.4us of continuous matmul activity to
    # lift the clock gate to 2.4 GHz; identity transposes don't count.
    warm_src = singles.tile([128, 512], bf16, tag="warm_src")
    nc.vector.memset(warm_src[:], 0.0)

    def warm(n):
        # gap fillers park in PV slot 0; ordered against any overlapping
        # PV chains by the tile framework (correctness-safe)
        for _ in range(n):
            nc.tensor.matmul(
                ops_tri[:, 0, :], lhsT=warm_src[:, 0:128],
                rhs=warm_src[:, 0:D + 1], start=True, stop=True,
            )

    # Sustained back-to-back burst: HAM needs ~3.4us of continuous PE
    # activity to lift the clock gate to 2.4 GHz. Alternating 512-col
    # targets in sA avoid WAW serialization between the dummies (sA's
    # first real use, buffer 1, overwrites every column before reading).
    for _i in range(14):
        nc.tensor.matmul(
            sA[:, (_i % 2) * 512:(_i % 2) * 512 + 512],
            lhsT=warm_src[:, 0:128], rhs=warm_src[:], start=True, stop=True,
        )

    ident = singles.tile([128, 128], bf16)
    masks.make_identity(nc, ident[:])
    keep = singles.tile([128, 128], bf16)
    masks.make_upper_triangular(nc, keep[:], val=1.0, diag=True)

    # ---- loads: per-head Q chunks just-in-time on the scalar HWDGE ring
    # (each HWDGE engine drives one ~125GB/s queue — loads must be spread
    # over the kernel, not front-loaded); K/V chunks on the sync ring.
    qnatp = ctx.enter_context(tc.tile_pool(name="qnatp", bufs=3))
    qnat = [None] * QH
    _qld_done = set()

    def qld(h, c):
        if (h, c) in _qld_done:
            return
        _qld_done.add((h, c))
        if qnat[h] is None:
            qnat[h] = qnatp.tile([128, NT, D], f32, tag="qnat", name="qnat")
        cs = slice(c * 4, (c + 1) * 4)
        qhr = q[:, h * D:(h + 1) * D].rearrange("(t p) d -> p t d", p=128)
        nc.scalar.dma_start(out=qnat[h][:, cs, :], in_=qhr[:, cs, :])

    kT = singles.tile([128, SEQ], bf16, tag="kT")
    knat = singles.tile([128, NT, 128], f32, tag="knat")
    knat_bf = singles.tile([128, NT, 128], bf16, tag="knat_bf")
    kr = k.rearrange("(t p) d -> p t d", p=128)
    vp = singles.tile([128, NT, D + 1], bf16)
    vnat = singles.tile([128, NT, 128], f32, tag="vnat")
    vr = v.rearrange("(t p) d -> p t d", p=128)
    nc.vector.memset(vp[:, :, D:D + 1], 1.0)

    qT = [
        singles.tile([128, SEQ], bf16, tag=f"qT{h}", name=f"qT{h}")
        for h in range(QH)
    ]

    # heads 1-3: Q^T via background DMA engines (SWDGE fp32->bf16 cast to
    # a DRAM scratch, then XBAR-transpose into SBUF), staggered into idle
    # fabric windows so they never compete with critical loads.
    q_sc = {h: nc.dram_tensor(f"q_sc{h}", [SEQ, D], bf16) for h in (1, 2, 3)}

    def qcast(h):
        nc.gpsimd.dma_start(out=q_sc[h][:, :], in_=q[:, h * D:(h + 1) * D])

    def qtrans(h):
        nc.sync.dma_start(out=qT[h][:, :], in_=q_sc[h][:, :], transpose=True)

    def qprep_chunk(h, c):
        """Cast + PE-transpose one 4-tile chunk of head h's Q (load must
        already have been issued via qld)."""
        cs = slice(c * 4, (c + 1) * 4)
        qbf = qbfp.tile([128, 4, 128], bf16, tag="qbf", name="qbf")
        nc.vector.tensor_copy(qbf[:], qnat[h][:, cs, :])
        for t in range(4):
            pst = tps[:, t % 2, :]
            nc.tensor.transpose(pst, qbf[:, t, :], ident[:])
            nc.vector.tensor_copy(
                qT[h][:, (c * 4 + t) * 128:(c * 4 + t + 1) * 128], pst
            )
        if c == 3:
            qnat[h] = None  # release the fp32 staging tile slot

    def kchunk(c):
        cs = slice(c * 4, (c + 1) * 4)
        nc.sync.dma_start(out=knat[:, cs, :], in_=kr[:, cs, :])
        nc.vector.tensor_copy(knat_bf[:, cs, :], knat[:, cs, :])
        for t in range(c * 4, (c + 1) * 4):
            pst = tps[:, t % 2, :]
            nc.tensor.transpose(pst, knat_bf[:, t, :], ident[:])
            nc.vector.tensor_copy(kT[:, t * 128:(t + 1) * 128], pst)

    def vchunk(c):
        cs = slice(c * 8, (c + 1) * 8)
        nc.sync.dma_start(out=vnat[:, cs, :], in_=vr[:, cs, :])
        nc.vector.tensor_copy(vp[:, cs, 0:D], vnat[:, cs, :])

    # Lazy head-0 prep, emitted just-in-time from inside the buffer walk
    prep_state = {"k": 0, "q0": 0}

    def need_k(kt):
        while prep_state["k"] * 4 <= kt:
            c = prep_state["k"]
            kchunk(c)
            warm(1)
            if c < 2:
                vchunk(c)
            prep_state["k"] += 1

    def need_q0(qhi):
        while prep_state["q0"] * 512 < qhi:
            c = prep_state["q0"]
            qld(0, c)
            if c + 1 < 4:
                qld(0, c + 1)  # stay one load ahead of the transposes
            qprep_chunk(0, c)
            warm(1)
            prep_state["q0"] += 1

    # heads 1-3 Q arrives via the background DMA path; casts are issued a
    # full head ahead of their transpose/use
    QPREP_EVENTS = {
        (0, 3): lambda: qcast(1),
        (0, 9): lambda: qtrans(1),
        (1, 3): lambda: qcast(2),
        (1, 9): lambda: qtrans(2),
        (2, 3): lambda: qcast(3),
        (2, 9): lambda: qtrans(3),
    }

    def emit_pv(h, qt, pT, osb):
        """O[qt] = sum_k2 (P^T slice).T @ [V | 1], then normalize + store."""
        ops = ops_tri[:, qt % 3, :]
        qtb = qt // 2
        for k2 in range(qt + 1):
            c0 = RSTART[(qtb, k2)] + (
                128 if (qt % 2 == 1 and k2 < 2 * qtb + 1) else 0
            )
            nc.tensor.matmul(
                ops,
                lhsT=pT[:, c0:c0 + 128],
                rhs=vp[:, k2, :],
                start=(k2 == 0),
                stop=(k2 == qt),
            )
        rec = opool.tile([128, 1], f32, tag="rec")
        nc.vector.reciprocal(rec[:], ops[:, D:D + 1])
        nc.vector.tensor_scalar_mul(osb[:, qt % 2, :], ops[:, 0:D], rec[:])
        if qt % 2 == 1:
            qb = qt // 2
            nc.sync.dma_start(
                out=out[qb * 256:(qb + 1) * 256, h * D:(h + 1) * D].rearrange(
                    "(j p) d -> p j d", p=128
                ),
                in_=osb[:],
            )

    # Pending-PV queue, flowing across head boundaries.
    pvq = []
    pv_state = {}

    def pop_pv():
        h2, qt2, pT2 = pvq.pop(0)
        st = pv_state.setdefault(h2, {})
        if qt2 % 2 == 0:
            st["osb"] = opool.tile([128, 2, D], f32, tag="osb", name="osb")
        emit_pv(h2, qt2, pT2, st["osb"])

    # per-head PV lag (in query tiles): small during head 0 so PV work
    # flows from the start (keeps the HAM clock gate engaged); small in
    # the last head so the post-loop drain tail is short
    HLAG = [1, 3, 3, 1]

    _ri = {"i": 0}

    def region_of(c):
        # REGIONS is walked strictly left-to-right within each head
        i = _ri["i"]
        while i + 1 < len(REGIONS) and REGIONS[i + 1][0] <= c:
            i += 1
        _ri["i"] = i
        return REGIONS[i]

    for h in range(QH):
        pT = ppool.tile([128, PCOLS], bf16, tag="pT")
        next_row = 0  # next query-pair row to mark PV-ready
        _ri["i"] = 0
        for bi, (b0, bsz, which) in enumerate(BUFS):
            ev = QPREP_EVENTS.get((h, bi))
            if ev is not None:
                ev()
            # drain PV backlog down to the per-head lag
            while len(pvq) > HLAG[h]:
                pop_pv()
            sbuf_tile = sA if which == 0 else sB
            # exact-causal QK chunks packed into this psum buffer
            c = b0
            while c < b0 + bsz:
                r0, rw, kt, qs = region_of(c)
                qoff = qs + (c - r0)  # query index of col c
                step = min(
                    512 - (c - b0) % 512,  # psum bank grid
                    r0 + rw - c,           # region end
                    b0 + bsz - c,          # buffer end
                )
                if h == 0:
                    need_k(kt)
                    need_q0(qoff + step)
                nc.tensor.matmul(
                    sbuf_tile[:, c - b0:c - b0 + step],
                    lhsT=kT[:, kt * 128:(kt + 1) * 128],
                    rhs=qT[h][:, qoff:qoff + step],
                    start=True,
                    stop=True,
                )
                c += step
            # one wide exp for the whole buffer
            nc.scalar.activation(
                pT[:, b0:b0 + bsz], sbuf_tile[:, 0:bsz], Exp, scale=SCALE
            )
            # rows completed by this buffer: mask the two diagonal tiles
            # (on GpSimd), then queue their query tiles for PV
            while next_row < NT // 2 and ROWEND[next_row] <= b0 + bsz:
                qtb = next_row
                for mc in (RSTART[(qtb, 2 * qtb)], RSTART[(qtb, 2 * qtb + 1)]):
                    nc.gpsimd.tensor_mul(
                        pT[:, mc:mc + 128], pT[:, mc:mc + 128], keep[:]
                    )
                pvq.append((h, 2 * qtb, pT))
                pvq.append((h, 2 * qtb + 1, pT))
                next_row += 1
    while pvq:
        pop_pv()


def _build():
    import concourse.mybir as mybir
    import concourse.tile as tile
    from concourse import bacc
    from contextlib import ExitStack

    nc = bacc.Bacc()
    q = nc.declare_dram_parameter("q", [SEQ, QH * D], mybir.dt.float32, isOutput=False)
    k = nc.declare_dram_parameter("k", [SEQ, D], mybir.dt.float32, isOutput=False)
    v = nc.declare_dram_parameter("v", [SEQ, D], mybir.dt.float32, isOutput=False)
    out = nc.declare_dram_parameter("out", [SEQ, QH * D], mybir.dt.float32, isOutput=True)

    with tile.TileContext(nc) as tc:
        with ExitStack() as ctx:
            _emit(ctx, tc, q[:], k[:], v[:], out[:])
    nc.compile()
    return nc


def _get_nc():
    global _NC
    if _NC is None:
        _NC = _build()
    return _NC


def _ensure_ntff_hook():
    """The agent image's antenv lacks axon_hooks; shim it so trace=True works."""
    import sys
    import types

    if "antenv.axon_hooks" in sys.modules:
        return
    try:
        import antenv
        from trn_agent_boot.trn_boot import _ntff_profile_via_ctypes
    except ImportError:
        return
    mod = types.ModuleType("antenv.axon_hooks")
    hook = [None]
    mod.set_axon_ntff_profile_hook = lambda h: hook.__setitem__(0, h)
    mod.get_axon_ntff_profile_hook = lambda: hook[0]
    sys.modules["antenv.axon_hooks"] = mod
    antenv.axon_hooks = mod
    mod.set_axon_ntff_profile_hook(_ntff_profile_via_ctypes("/opt/axon/libaxon_pjrt.so"))


def _run(q, k, v, trace=False):
    from concourse.bass_utils import run_bass_kernel_spmd

    if trace:
        _ensure_ntff_hook()
    nc = _get_nc()
    in_maps = []
    for i in range(N_CORES):
        in_maps.append(
            {
                "q": np.ascontiguousarray(q[:, i * QH * D:(i + 1) * QH * D]).astype(np.float32, copy=False),
                "k": np.ascontiguousarray(k[:, i * D:(i + 1) * D]).astype(np.float32, copy=False),
                "v": np.ascontiguousarray(v[:, i * D:(i + 1) * D]).astype(np.float32, copy=False),
            }
        )
    res = run_bass_kernel_spmd(nc, in_maps, core_ids=list(range(N_CORES)), trace=trace)
    full = np.concatenate([res.results[i]["out"] for i in range(N_CORES)], axis=1)
    return full.astype(np.float32, copy=False), res


def kernel(q, k, v):
    out, _ = _run(q, k, v, trace=False)
    return out
